# revision 8
# baseline (speedup 1.0000x reference)
"""Trainium2 Bass kernel: single-head causal attention head layer.

Reference computation (per batch b):
    q = x[b] @ Wq; k = x[b] @ Wk; v = x[b] @ Wv        # [S, H], H=64
    w = softmax_causal(q @ k.T * E**-0.5)              # [S, S]
    out[b] = w @ v                                     # [S, H]

Shapes: x (8, 2048, 1024) f32, Wq/Wk/Wv (1024, 64) f32 -> out (8, 2048, 64) f32.

Sharding: data-parallel over batch, one batch per NeuronCore (8 cores).

Device algorithm (per core), all matmuls bf16 with fp32 PSUM accumulation:
  1. Projections, pipelined against the x^T DMA stream (per 128-row
     e-tile): [Wq|Wk] stationary -> qk^T psum [128, 2048] (rows 0:64 q^T,
     64:128 k^T); Wv stationary -> v^T psum [64, 2048]. PE chases DMA.
  2. k^T moved to a base-0 SBUF tile via SBUF->SBUF DMA (PE needs lhsT and
     rhs on the same base partition). v^T PE-transposed into 16 v tiles
     [128, 65] with a ones column (row 64 makes softmax denominators fall
     out of the AV matmul for free).
  3. Scores transposed: S^T[j,i] = k_j . q_i, keys on partitions, so the
     softmax denominator is a partition-dim sum -> folded into step 5.
  4. exp on ScalarE with scale=E**-0.5, 1024-wide chunks. No
     max-subtraction: scaled scores are N(0, 0.0625), exp is safe.
     Causal masking: block-skip + multiplicative 0/1 bf16 mask on the
     diagonal chunk.
  5. O^T_aug[0:64,i] = sum_j v[j,h] P^T[j,i]; row 64 = denominators.
  6. PE-transpose O^T_aug 128-col slices -> [128, 65]; reciprocal of
     col 64; scale; DMA out fp32.
"""

import numpy as np
import ml_dtypes

BATCH = 8
SEQ = 2048
EMBED = 1024
HEAD = 64
N_CORES = 8
SCALE = float(EMBED) ** -0.5  # 0.03125

ST = SEQ // 128  # 16 seq tiles
ET = EMBED // 128  # 8 embed tiles

_CACHE = {}


def _build_program():
    import concourse.mybir as mybir
    from concourse import bacc
    from concourse.tile import TileContext

    f32 = mybir.dt.float32
    bf16 = mybir.dt.bfloat16
    EXP = mybir.ActivationFunctionType.Exp

    nc = bacc.Bacc("TRN2", target_bir_lowering=False, debug=False,
                   num_devices=N_CORES)

    xT = nc.declare_dram_parameter("xT", [EMBED, SEQ], bf16, isOutput=False)
    wqk = nc.declare_dram_parameter("wqk", [128, ET, 128], bf16, isOutput=False)
    wv = nc.declare_dram_parameter("wv", [128, ET, HEAD], bf16, isOutput=False)
    masks = nc.declare_dram_parameter("masks", [128, 8, 1024], bf16,
                                      isOutput=False)
    ident65 = nc.declare_dram_parameter("ident65", [HEAD + 1, HEAD + 1], f32,
                                        isOutput=False)
    ident64 = nc.declare_dram_parameter("ident64", [HEAD, HEAD], bf16,
                                        isOutput=False)
    out = nc.declare_dram_parameter("out", [SEQ, HEAD], f32, isOutput=True)

    with TileContext(nc) as tc:
        with (
            tc.tile_pool(name="persist", bufs=1) as persist,
            tc.tile_pool(name="xtp", bufs=1) as xtp,
            tc.tile_pool(name="vtiles", bufs=1) as vtiles,
            tc.tile_pool(name="psb", bufs=4) as psb,
            tc.tile_pool(name="osb", bufs=3) as osb,
            tc.tile_pool(name="rsb", bufs=3) as rsb,
        ):
            # ---- load weights/constants first (small) ----
            wqk_sb = persist.tile([128, ET, 128], bf16)
            nc.sync.dma_start(out=wqk_sb[:], in_=wqk[:])
            wv_sb = persist.tile([128, ET, HEAD], bf16)
            nc.sync.dma_start(out=wv_sb[:], in_=wv[:])
            mask_sb = persist.tile([128, 8, 1024], bf16)
            nc.sync.dma_start(out=mask_sb[:], in_=masks[:])
            id65_sb = persist.tile([HEAD + 1, HEAD + 1], f32)
            nc.sync.dma_start(out=id65_sb[:], in_=ident65[:])
            id64_sb = persist.tile([HEAD, HEAD], bf16)
            nc.sync.dma_start(out=id64_sb[:], in_=ident64[:])

            qk_sb = persist.tile([128, SEQ], bf16)  # rows 0:64 qT, 64:128 kT
            kt_sb = persist.tile([64, SEQ], bf16)  # kT at base partition 0
            vt_sb = persist.tile([64, SEQ], bf16)  # vT
            v_sbs = []
            for s in range(ST):
                v_sbs.append(vtiles.tile([128, HEAD + 1], bf16,
                                         name=f"v{s}", tag=f"v{s}"))
            ot_sb = persist.tile([HEAD + 1, SEQ], f32)

            # ---- Phase B: projections, pipelined against the xT DMA ----
            with tc.tile_pool(name="ps_b", bufs=1, space="PSUM") as ps_b:
                qk_ps = ps_b.tile([128, SEQ], f32)
                vt_ps = ps_b.tile([64, SEQ], f32)
                xts = []
                for e in range(ET):
                    xt_e = xtp.tile([128, SEQ], bf16, name=f"xt{e}",
                                    tag=f"xt{e}")
                    xts.append(xt_e)
                    nc.sync.dma_start(out=xt_e[:],
                                      in_=xT[128 * e:128 * (e + 1), :])
                    for c in range(SEQ // 512):
                        nc.tensor.matmul(
                            qk_ps[:, 512 * c:512 * (c + 1)],
                            lhsT=wqk_sb[:, e, :],
                            rhs=xt_e[:, 512 * c:512 * (c + 1)],
                            start=(e == 0), stop=(e == ET - 1),
                        )
                    for c in range(SEQ // 512):
                        nc.tensor.matmul(
                            vt_ps[:, 512 * c:512 * (c + 1)],
                            lhsT=wv_sb[:, e, :],
                            rhs=xt_e[:, 512 * c:512 * (c + 1)],
                            start=(e == 0), stop=(e == ET - 1),
                        )
                for c in range(SEQ // 512):
                    nc.vector.tensor_copy(qk_sb[:, 512 * c:512 * (c + 1)],
                                          qk_ps[:, 512 * c:512 * (c + 1)])
                for c in range(SEQ // 512):
                    nc.vector.tensor_copy(vt_sb[:, 512 * c:512 * (c + 1)],
                                          vt_ps[:, 512 * c:512 * (c + 1)])
                # PE matmul needs lhsT/rhs on the same base partition:
                # shift kT (partitions 64:128) down to a base-0 tile.
                nc.sync.dma_start(out=kt_sb[:], in_=qk_sb[64:128, :])

            # ---- v layout fix: PE-transpose v^T 128-col slices ----
            with tc.tile_pool(name="ps_vt", bufs=2, space="PSUM") as ps_vt:
                for s in range(ST):
                    v_ps = ps_vt.tile([128, HEAD], bf16, tag="vps")
                    nc.tensor.transpose(
                        v_ps[:], vt_sb[:, 128 * s:128 * (s + 1)], id64_sb[:])
                    nc.vector.memset(v_sbs[s][:, HEAD:HEAD + 1], 1.0)
                    nc.vector.tensor_copy(v_sbs[s][:, 0:HEAD], v_ps[:])

            # ---- Phase D: attention ----
            with tc.tile_pool(name="ps_o", bufs=1, space="PSUM") as ps_o:
                o_ps = ps_o.tile([HEAD + 1, SEQ], f32)

                with tc.tile_pool(name="ps_s", bufs=2, space="PSUM") as ps_s:
                    for j in range(ST):
                        kT = kt_sb[:, 128 * j:128 * (j + 1)]
                        c0 = j // 8
                        lo = 128 * (j % 8)
                        for cc in range(c0, 2):
                            klo = lo if cc == c0 else 0
                            base = 1024 * cc
                            s_ps = ps_s.tile([128, 1024], f32, tag="sps")
                            for h in (0, 512):
                                a = max(klo, h)
                                if a < h + 512:
                                    nc.tensor.matmul(
                                        s_ps[:, a:h + 512],
                                        lhsT=kT,
                                        rhs=qk_sb[0:64, base + a:base + h + 512],
                                        start=True, stop=True,
                                    )
                            p_sb = psb.tile([128, 1024], bf16, tag="psb")
                            if 0 < klo < 512:
                                nc.vector.memset(p_sb[:, 0:klo], 0.0)
                            elif klo > 512:
                                nc.vector.memset(p_sb[:, 512:klo], 0.0)
                            nc.scalar.activation(p_sb[:, klo:1024],
                                                 s_ps[:, klo:1024],
                                                 EXP, scale=SCALE)
                            if cc == c0:
                                nc.vector.tensor_mul(
                                    p_sb[:, klo:1024], p_sb[:, klo:1024],
                                    mask_sb[:, j % 8, klo:1024])
                            for hh in (0, 1):
                                g = 2 * cc + hh
                                if j > 4 * g + 3:
                                    continue  # fully above causal diagonal
                                nc.tensor.matmul(
                                    o_ps[:, 512 * g:512 * (g + 1)],
                                    lhsT=v_sbs[j][:],
                                    rhs=p_sb[:, 512 * hh:512 * (hh + 1)],
                                    start=(j == 0), stop=(j == 4 * g + 3),
                                )

                    for c in range(SEQ // 512):
                        nc.vector.tensor_copy(ot_sb[:, 512 * c:512 * (c + 1)],
                                              o_ps[:, 512 * c:512 * (c + 1)])

                # ---- Phase E: transpose + normalize + store ----
                with tc.tile_pool(name="ps_t", bufs=2, space="PSUM") as ps_t:
                    for s in range(ST):
                        t_ps = ps_t.tile([128, HEAD + 1], f32, tag="tps")
                        nc.tensor.transpose(
                            t_ps[:], ot_sb[:, 128 * s:128 * (s + 1)],
                            id65_sb[:])
                        recip = rsb.tile([128, 1], f32, tag="recip")
                        nc.vector.reciprocal(recip[:],
                                             t_ps[:, HEAD:HEAD + 1])
                        o_sb = osb.tile([128, HEAD], f32, tag="osb")
                        nc.vector.tensor_scalar_mul(o_sb[:], t_ps[:, 0:HEAD],
                                                    recip[:])
                        nc.sync.dma_start(
                            out=out[128 * s:128 * (s + 1), :], in_=o_sb[:])

    nc.compile()
    return nc


def _get_program():
    if "nc" not in _CACHE:
        _CACHE["nc"] = _build_program()
    return _CACHE["nc"]


def _host_inputs(x, Wq, Wk, Wv):
    bf16 = ml_dtypes.bfloat16
    # x^T per batch: [E, S] contiguous, bf16
    xT = np.ascontiguousarray(x.transpose(0, 2, 1)).astype(bf16)
    # [Wq | Wk] -> [128, ET, 128] (partition = embed % 128)
    wqk = np.concatenate([Wq, Wk], axis=1).astype(bf16)  # [E, 128]
    wqk = np.ascontiguousarray(
        wqk.reshape(ET, 128, 128).transpose(1, 0, 2))  # [128, ET, 128]
    wv = np.ascontiguousarray(
        Wv.astype(bf16).reshape(ET, 128, HEAD).transpose(1, 0, 2))
    # masks[m][x, y] = 1.0 iff y - x - 128*m >= 0
    xx = np.arange(128)[:, None]
    yy = np.arange(1024)[None, :]
    masks = np.stack([(yy - xx - 128 * m >= 0) for m in range(8)],
                     axis=1).astype(bf16)  # [128, 8, 1024]
    ident65 = np.eye(HEAD + 1, dtype=np.float32)
    ident64 = np.eye(HEAD, dtype=bf16)
    return xT, wqk, wv, masks, ident65, ident64


def kernel(x, Wq, Wk, Wv):
    from concourse.bass_utils import run_bass_kernel_spmd

    nc = _get_program()
    xT, wqk, wv, masks, ident65, ident64 = _host_inputs(x, Wq, Wk, Wv)
    in_maps = [
        {"xT": xT[b], "wqk": wqk, "wv": wv, "masks": masks,
         "ident65": ident65, "ident64": ident64}
        for b in range(BATCH)
    ]
    res = run_bass_kernel_spmd(nc, in_maps, list(range(N_CORES)))
    out = np.stack([np.asarray(res.results[b]["out"]) for b in range(BATCH)])
    return out.astype(np.float32)


# revision 9
# speedup vs baseline: 1.1511x; 1.1511x over previous
"""Trainium2 Bass kernel: single-head causal attention head layer.

Reference computation (per batch b):
    q = x[b] @ Wq; k = x[b] @ Wk; v = x[b] @ Wv        # [S, H], H=64
    w = softmax_causal(q @ k.T * E**-0.5)              # [S, S]
    out[b] = w @ v                                     # [S, H]

Shapes: x (8, 2048, 1024) f32, Wq/Wk/Wv (1024, 64) f32 -> out (8, 2048, 64) f32.

Sharding: data-parallel over batch, one batch per NeuronCore (8 cores).

Device algorithm (per core), all matmuls bf16 with fp32 PSUM accumulation:
  1. Projections pipelined against the x^T DMA stream (per 128-row e-tile):
     [Wq|Wk] stationary -> qk^T psum [128, 2048] (rows 0:64 q^T, 64:128 k^T);
     Wv stationary -> v^T psum [64, 2048].
  2. k^T copied to rows 0:64 of a [128, 2048] tile whose rows 64:128 are
     zero. Scores can then contract over K=128 against the full qk tile
     (zero weights kill the k^T rows): full PE array activity, which keeps
     the HAM clock gate at 2.4 GHz. v^T is PE-transposed into 16 tiles
     v_aug [128, 128]: cols 0:64 = v, col 64 = ones (row sums of P fall out
     of the AV matmul for free), cols 65:128 = zero (pads M to 128).
  3. Scores transposed: S^T[j,i] = k_j . q_i, keys on partitions, so the
     softmax denominator is a partition-dim sum -> folded into step 5.
  4. exp on ScalarE with scale=E**-0.5, 1024-wide chunks. No
     max-subtraction: scaled scores are N(0, 0.0625), exp is safe. Causal
     masking: block skip + multiplicative 0/1 bf16 mask on diagonal chunks.
  5. O^T_aug[h,i] = sum_j v_aug[j,h] P^T[j,i] into four 512-col psum
     region tiles; row 64 = denominators. When a region's last j arrives,
     it is finalized immediately (overlaps the remaining attention):
  6. PE-transpose 128-col slices -> [128, 128]; reciprocal of col 64;
     scale cols 0:64; DMA out fp32.
"""

import numpy as np
import ml_dtypes

BATCH = 8
SEQ = 2048
EMBED = 1024
HEAD = 64
N_CORES = 8
SCALE = float(EMBED) ** -0.5  # 0.03125

ST = SEQ // 128  # 16 seq tiles
ET = EMBED // 128  # 8 embed tiles

_CACHE = {}


def _build_program():
    import concourse.mybir as mybir
    from concourse import bacc
    from concourse.tile import TileContext

    f32 = mybir.dt.float32
    bf16 = mybir.dt.bfloat16
    EXP = mybir.ActivationFunctionType.Exp

    nc = bacc.Bacc("TRN2", target_bir_lowering=False, debug=False,
                   num_devices=N_CORES)

    xT = nc.declare_dram_parameter("xT", [EMBED, SEQ], bf16, isOutput=False)
    wqk = nc.declare_dram_parameter("wqk", [128, ET, 128], bf16, isOutput=False)
    wv = nc.declare_dram_parameter("wv", [128, ET, HEAD], bf16, isOutput=False)
    masks = nc.declare_dram_parameter("masks", [128, 8, 1024], bf16,
                                      isOutput=False)
    ident = nc.declare_dram_parameter("ident", [128, 128], f32, isOutput=False)
    ident64 = nc.declare_dram_parameter("ident64", [HEAD, HEAD], bf16,
                                        isOutput=False)
    out = nc.declare_dram_parameter("out", [SEQ, HEAD], f32, isOutput=True)

    with TileContext(nc) as tc:
        with (
            tc.tile_pool(name="persist", bufs=1) as persist,
            tc.tile_pool(name="xtp", bufs=1) as xtp,
            tc.tile_pool(name="vtiles", bufs=1) as vtiles,
            tc.tile_pool(name="psb", bufs=4) as psb,
            tc.tile_pool(name="osb", bufs=4) as osb,
            tc.tile_pool(name="rsb", bufs=4) as rsb,
        ):
            # ---- weights/constants; two issue streams (sync + gpsimd) ----
            wqk_sb = persist.tile([128, ET, 128], bf16)
            nc.sync.dma_start(out=wqk_sb[:], in_=wqk[:])
            wv_sb = persist.tile([128, ET, HEAD], bf16)
            nc.gpsimd.dma_start(out=wv_sb[:], in_=wv[:])
            id64_sb = persist.tile([HEAD, HEAD], bf16)
            nc.gpsimd.dma_start(out=id64_sb[:], in_=ident64[:])

            # preload the exp table set so ACT_TABLE_LOAD overlaps the DMAs
            warm_sb = persist.tile([128, 1], f32)
            nc.vector.memset(warm_sb[:], 0.0)
            nc.scalar.activation(warm_sb[:], warm_sb[:], EXP, scale=1.0)

            qk_sb = persist.tile([128, SEQ], bf16)  # rows 0:64 qT, 64:128 kT
            kt2_sb = persist.tile([128, SEQ], bf16)  # rows 0:64 kT, rest 0
            nc.vector.memset(kt2_sb[64:128, :], 0.0)
            vt_sb = persist.tile([64, SEQ], bf16)  # vT
            v_sbs = []
            for s in range(ST):
                v_sbs.append(vtiles.tile([128, 128], bf16,
                                         name=f"v{s}", tag=f"v{s}"))
            ot_sb = persist.tile([128, SEQ], f32)

            # ---- Phase B: projections, pipelined against the xT DMA ----
            with tc.tile_pool(name="ps_b", bufs=1, space="PSUM") as ps_b:
                qk_ps = ps_b.tile([128, SEQ], f32)
                vt_ps = ps_b.tile([64, SEQ], f32)
                for e in range(ET):
                    xt_e = xtp.tile([128, SEQ], bf16, name=f"xt{e}",
                                    tag=f"xt{e}")
                    eng = nc.sync if e % 2 == 0 else nc.gpsimd
                    eng.dma_start(out=xt_e[:],
                                  in_=xT[128 * e:128 * (e + 1), :])
                    for c in range(SEQ // 512):
                        nc.tensor.matmul(
                            qk_ps[:, 512 * c:512 * (c + 1)],
                            lhsT=wqk_sb[:, e, :],
                            rhs=xt_e[:, 512 * c:512 * (c + 1)],
                            start=(e == 0), stop=(e == ET - 1),
                        )
                    for c in range(SEQ // 512):
                        nc.tensor.matmul(
                            vt_ps[:, 512 * c:512 * (c + 1)],
                            lhsT=wv_sb[:, e, :],
                            rhs=xt_e[:, 512 * c:512 * (c + 1)],
                            start=(e == 0), stop=(e == ET - 1),
                        )
                # masks are only needed from the attention phase on
                mask_sb = persist.tile([128, 8, 1024], bf16)
                nc.gpsimd.dma_start(out=mask_sb[:], in_=masks[:])
                id_sb = persist.tile([128, 128], f32)
                nc.gpsimd.dma_start(out=id_sb[:], in_=ident[:])

                for c in range(SEQ // 512):
                    nc.vector.tensor_copy(qk_sb[:, 512 * c:512 * (c + 1)],
                                          qk_ps[:, 512 * c:512 * (c + 1)])
                for c in range(SEQ // 512):
                    nc.vector.tensor_copy(vt_sb[:, 512 * c:512 * (c + 1)],
                                          vt_ps[:, 512 * c:512 * (c + 1)])
                # shift kT (partitions 64:128) down to base partition 0
                nc.sync.dma_start(out=kt2_sb[0:64, :], in_=qk_sb[64:128, :])

            # ---- v layout fix: PE-transpose v^T 128-col slices ----
            with tc.tile_pool(name="ps_vt", bufs=2, space="PSUM") as ps_vt:
                for s in range(ST):
                    v_ps = ps_vt.tile([128, HEAD], bf16, tag="vps")
                    nc.tensor.transpose(
                        v_ps[:], vt_sb[:, 128 * s:128 * (s + 1)], id64_sb[:])
                    nc.vector.memset(v_sbs[s][:, HEAD:HEAD + 1], 1.0)
                    nc.vector.memset(v_sbs[s][:, HEAD + 1:128], 0.0)
                    nc.vector.tensor_copy(v_sbs[s][:, 0:HEAD], v_ps[:])

            # ---- Phase D: attention, with per-region finalization ----
            with (
                tc.tile_pool(name="ps_o", bufs=1, space="PSUM") as ps_o,
                tc.tile_pool(name="ps_s", bufs=2, space="PSUM") as ps_s,
            ):
                o_regs = []
                for g in range(4):
                    o_regs.append(ps_o.tile([128, 512], f32,
                                            name=f"oreg{g}", tag=f"oreg{g}"))

                def finalize_region(g):
                    # region g covers queries [512g, 512(g+1))
                    nc.vector.tensor_copy(ot_sb[:, 512 * g:512 * (g + 1)],
                                          o_regs[g][:])
                    for ss in range(4):
                        s = 4 * g + ss
                        t_ps = ps_s.tile([128, 128], f32, tag="sps")
                        nc.tensor.transpose(
                            t_ps[:], ot_sb[:, 128 * s:128 * (s + 1)],
                            id_sb[:])
                        recip = rsb.tile([128, 1], f32, tag="recip")
                        nc.vector.reciprocal(recip[:],
                                             t_ps[:, HEAD:HEAD + 1])
                        o_sb = osb.tile([128, HEAD], f32, tag="osb")
                        nc.vector.tensor_scalar_mul(o_sb[:], t_ps[:, 0:HEAD],
                                                    recip[:])
                        nc.gpsimd.dma_start(
                            out=out[128 * s:128 * (s + 1), :], in_=o_sb[:])

                for j in range(ST):
                    c0 = j // 8
                    lo = 128 * (j % 8)
                    for cc in range(c0, 2):
                        klo = lo if cc == c0 else 0
                        base = 1024 * cc
                        s_ps = ps_s.tile([128, 1024], f32, tag="sps")
                        for h in (0, 512):
                            a = max(klo, h)
                            if a < h + 512:
                                nc.tensor.matmul(
                                    s_ps[:, a:h + 512],
                                    lhsT=kt2_sb[:, 128 * j:128 * (j + 1)],
                                    rhs=qk_sb[:, base + a:base + h + 512],
                                    start=True, stop=True,
                                )
                        p_sb = psb.tile([128, 1024], bf16, tag="psb")
                        if 0 < klo < 512:
                            nc.vector.memset(p_sb[:, 0:klo], 0.0)
                        elif klo > 512:
                            nc.vector.memset(p_sb[:, 512:klo], 0.0)
                        nc.scalar.activation(p_sb[:, klo:1024],
                                             s_ps[:, klo:1024],
                                             EXP, scale=SCALE)
                        if cc == c0:
                            nc.vector.tensor_mul(
                                p_sb[:, klo:1024], p_sb[:, klo:1024],
                                mask_sb[:, j % 8, klo:1024])
                        for hh in (0, 1):
                            g = 2 * cc + hh
                            if j > 4 * g + 3:
                                continue  # fully above causal diagonal
                            nc.tensor.matmul(
                                o_regs[g][:],
                                lhsT=v_sbs[j][:],
                                rhs=p_sb[:, 512 * hh:512 * (hh + 1)],
                                start=(j == 0), stop=(j == 4 * g + 3),
                            )
                            if j == 4 * g + 3:
                                finalize_region(g)

    nc.compile()
    return nc


def _get_program():
    if "nc" not in _CACHE:
        _CACHE["nc"] = _build_program()
    return _CACHE["nc"]


def _host_inputs(x, Wq, Wk, Wv):
    bf16 = ml_dtypes.bfloat16
    # x^T per batch: [E, S] contiguous, bf16
    xT = np.ascontiguousarray(x.transpose(0, 2, 1)).astype(bf16)
    # [Wq | Wk] -> [128, ET, 128] (partition = embed % 128)
    wqk = np.concatenate([Wq, Wk], axis=1).astype(bf16)  # [E, 128]
    wqk = np.ascontiguousarray(
        wqk.reshape(ET, 128, 128).transpose(1, 0, 2))  # [128, ET, 128]
    wv = np.ascontiguousarray(
        Wv.astype(bf16).reshape(ET, 128, HEAD).transpose(1, 0, 2))
    # masks[m][x, y] = 1.0 iff y - x - 128*m >= 0
    xx = np.arange(128)[:, None]
    yy = np.arange(1024)[None, :]
    masks = np.stack([(yy - xx - 128 * m >= 0) for m in range(8)],
                     axis=1).astype(bf16)  # [128, 8, 1024]
    ident = np.eye(128, dtype=np.float32)
    ident64 = np.eye(HEAD, dtype=bf16)
    return xT, wqk, wv, masks, ident, ident64


def kernel(x, Wq, Wk, Wv):
    from concourse.bass_utils import run_bass_kernel_spmd

    nc = _get_program()
    xT, wqk, wv, masks, ident, ident64 = _host_inputs(x, Wq, Wk, Wv)
    in_maps = [
        {"xT": xT[b], "wqk": wqk, "wv": wv, "masks": masks,
         "ident": ident, "ident64": ident64}
        for b in range(BATCH)
    ]
    res = run_bass_kernel_spmd(nc, in_maps, list(range(N_CORES)))
    out = np.stack([np.asarray(res.results[b]["out"]) for b in range(BATCH)])
    return out.astype(np.float32)
